# revision 35
# baseline (speedup 1.0000x reference)
"""Trainium2 Bass kernel for ConvGlobalLocalCapsuleLayer.

Per-capsule 3x3 SAME conv (8 capsules, 16->128 ch) + 3 iterations of dynamic
routing (softmax over output capsules, squash nonlinearity).

Sharding: data-parallel over batch B=32 across 8 cores (4 images/core),
weights replicated. No cross-core communication.

v2 design (transposed routing). All capsule columns use (f,c) order so that
per-(i,c)/per-(c) multipliers broadcast with unit innermost stride (DVE 2x).

Per-core pipeline (positions chunked 512 = 16 h-rows):
  - load x natural [pos,128ch], PE-transpose, cast fp16 into x_sb
    [(i,cin), padded 34x34 geometry]; x3 im2col over dy (as before)
  - conv: per (i,dx) K=96 fp16 matmul accumulating over dx -> votes_o
    [fc, (i,pos)] fp16 SBUF; S = sum_i votes via 9 K=128 matmuls (w_s
    pre-scaled 1/8)
  - votes transposed per (i,sub) by PE into votes_t [pos, sub, i, f, c]
    fp16 SBUF; S transposed likewise (stays in PSUM fp16)
  - routing iterations mostly in transposed layout:
      squash: ACT square, DVE reduce over f -> s2[p,32], ln/exp sqrt trick
      (stays in one ACT table set: no table reloads), small fp32 chain,
      2x fp16 apply-mult
      agreement: rv = votes_o * act_o (act transposed back by PE, read
      straight from PSUM fp16), f-reduce via 8 constant ei matmuls into
      PSUM logits accumulated across iterations
      softmax: ACT exp (fp16, shifted by -2), Z via oneii matmul,
      reciprocal on DVE, normalize on GPSIMD
      route weighting: route_t broadcast over f (free AP view), 2x fp16
      multiply; i-sum as add-tree (first level on GPSIMD)
  - output: direct DMA of act_t [pos, (f,c)]; host permutes (f,c)->(c,f)
"""
import sys

sys.path.insert(0, "/opt/trn_rl_repo")
sys.path.insert(0, "/root/.axon_site/_ro/trn_rl_repo")

import numpy as np
from concourse import bacc, mybir, tile
from concourse.bass_utils import run_bass_kernel_spmd

# Steer the activation-table chooser to the one set that contains every
# function this kernel uses (copy/square/exp/ln), so the ACT engine loads a
# single table instead of ping-ponging between exp-only and ln-only sets.
# Indices (= act_func_set_id) are preserved; only membership visibility for
# the chooser is filtered, so walrus still maps to the correct table.
_orig_gat = bacc.get_activation_tables


def _gat_one_set(arch):
    tabs = _orig_gat(arch)
    keep = "natural_log_exp_and_others"
    if keep not in tabs:
        return tabs
    mine = {mybir.ActivationFunctionType.Copy,
            mybir.ActivationFunctionType.Identity,
            mybir.ActivationFunctionType.Square,
            mybir.ActivationFunctionType.Exp,
            mybir.ActivationFunctionType.Ln}
    out = {}
    for name, funcs in tabs.items():
        out[name] = funcs if name == keep else (funcs - mine)
    return out


bacc.get_activation_tables = _gat_one_set

dt = mybir.dt
AF = mybir.ActivationFunctionType
OP = mybir.AluOpType
AX = mybir.AxisListType

N_CORES = 8
EPS = 1e-7
IMG, HH, WW, ICAPS, CIN, C, F = 4, 32, 32, 8, 16, 8, 16
CF = 128
PH, PW = 34, 34
IMGLEN = PH * PW            # 1156
GUARD = 34
NCHUNK = 8                  # 512-position chunks (16 h-rows each)
TAPS = [(dy, dx) for dy in (-1, 0, 1) for dx in (-1, 0, 1)]

_CACHE = {}


def _host_constants(W, b):
    """Constant matrices, all vote columns permuted to (f,c) order."""
    W = np.asarray(W, np.float32)
    b = np.asarray(b, np.float32)
    # (c,f) -> (f,c) column permutation: new_col f*C+c takes old col c*F+f
    perm = np.zeros(CF, np.int64)
    for c in range(C):
        for f in range(F):
            perm[f * C + c] = c * F + f
    Wp = W[..., perm]                       # [i, 3, 3, cin, (f,c)]
    cst = {}
    w_s = np.zeros((128, 9 * CF), np.float16)
    for t, (dy, dx) in enumerate(TAPS):
        w_s[:, t * CF:(t + 1) * CF] = (
            Wp[:, dy + 1, dx + 1, :, :].reshape(128, CF) * 0.125)
    cst["w_s"] = w_s
    w_c = np.zeros((96, ICAPS * 3 * CF), np.float16)
    for i in range(ICAPS):
        q = i % 2
        for dxi in range(3):
            blk = np.zeros((96, CF), np.float16)
            for dyi in range(3):
                blk[q * 48 + dyi * 16:q * 48 + dyi * 16 + 16] = Wp[i, dyi, dxi]
            w_c[:, (i * 3 + dxi) * CF:(i * 3 + dxi + 1) * CF] = blk
    cst["w_c"] = w_c
    # ei: contract votes rows (f,c) -> logits rows (i,c) per input capsule
    ei = np.zeros((CF, ICAPS * 64), np.float16)
    for i in range(ICAPS):
        for c in range(C):
            for f in range(F):
                ei[f * C + c, i * 64 + i * C + c] = 1.0
    cst["ei"] = ei
    oneii = np.zeros((64, 64), np.float32)
    for i in range(ICAPS):
        oneii[i * C:(i + 1) * C, i * C:(i + 1) * C] = 1.0
    cst["oneii"] = oneii
    cst["idm16"] = np.eye(128, dtype=np.float16)
    cst["idm32"] = np.eye(128, dtype=np.float32)
    # bias broadcast tile in (f,c) order, replicated across partitions
    b_fc = np.zeros((128, CF), np.float16)
    bj = b.reshape(C, F)
    for c in range(C):
        for f in range(F):
            b_fc[:, f * C + c] = bj[c, f]
    cst["b_fc"] = b_fc
    cst["epsv"] = np.full((128, 1), EPS, np.float32)
    cst["neg2"] = np.full((128, 1), -2.0, np.float32)
    return cst


_CONST_SPECS = [
    ("w_s", [128, 9 * CF], dt.float16),
    ("w_c", [96, ICAPS * 3 * CF], dt.float16),
    ("ei", [CF, ICAPS * 64], dt.float16),
    ("oneii", [64, 64], dt.float32r),
    ("idm16", [128, 128], dt.float16),
    ("idm32", [128, 128], dt.float32),
    ("b_fc", [128, CF], dt.float16),
    ("epsv", [128, 1], dt.float32),
    ("neg2", [128, 1], dt.float32),
]


def _build_program():
    nc = bacc.Bacc("TRN2", target_bir_lowering=False, debug=False)
    x_d = nc.dram_tensor("x", [IMG * HH * WW, 128], dt.float32,
                         kind="ExternalInput").ap()
    out_d = nc.dram_tensor("out", [IMG * HH * WW, 128], dt.float16,
                           kind="ExternalOutput").ap()
    cst_d = {n: nc.dram_tensor(n, sh, d, kind="ExternalInput").ap()
             for n, sh, d in _CONST_SPECS}

    with tile.TileContext(nc) as tc:
        with (
            tc.tile_pool(name="const", bufs=1) as cpool,
            tc.tile_pool(name="xbig", bufs=1) as xpool,
            tc.tile_pool(name="xnat", bufs=4) as npool,
            tc.tile_pool(name="votes", bufs=3) as vpool,
            tc.tile_pool(name="work", bufs=2) as wpool,
            tc.tile_pool(name="med", bufs=2) as mpool,
            tc.tile_pool(name="ps_vp", bufs=2, space="PSUM") as ps_vp,
            tc.tile_pool(name="ps_t16", bufs=2, space="PSUM") as ps_t16,
            tc.tile_pool(name="ps_L", bufs=3, space="PSUM") as ps_L,
            tc.tile_pool(name="ps_z", bufs=1, space="PSUM") as ps_z,
        ):
            cst = {}
            for n, sh, d in _CONST_SPECS:
                t = cpool.tile(sh, d, tag=n)
                nc.sync.dma_start(t[:], cst_d[n][:])
                cst[n] = t

            zbank = ps_z.tile([128, 512], dt.float32, tag="z")

            def zslice(ch):
                return zbank[:64, :]

            # -------- x load + transpose + cast fp16, split in halves ------
            HLEN = 2 * IMGLEN
            x_sbh = []
            x3h = []

            def xgeom2(ap):
                return ap.rearrange("p (im h w) -> p im h w", im=2, h=PH, w=PW)

            for half in range(2):
                xs = xpool.tile([128, 2 * GUARD + HLEN], dt.float16,
                                tag="x_sb", bufs=2)
                nc.gpsimd.memset(xs[:], 0.0)
                x_sbh.append(xs)
                for quad in range(4):
                    base = (half * 16 + quad * 4) * 128
                    xt = npool.tile([128, 4 * 128], dt.float32, tag="xnat")
                    src4 = x_d[base:base + 512, :].rearrange(
                        "(blk p) c -> p blk c", p=128)
                    nc.sync.dma_start(
                        xt[:].rearrange("p (blk c) -> p blk c", blk=4), src4)
                    for blk in range(4):
                        sub = quad * 4 + blk
                        img_loc, h0 = sub // 8, (sub % 8) * 4
                        tp = ps_vp.tile([128, 512], dt.float32, tag="vp")
                        nc.tensor.transpose(
                            tp[:, :128], xt[:, blk * 128:(blk + 1) * 128],
                            cst["idm32"][:])
                        dst = xgeom2(xs[:, GUARD:GUARD + HLEN])[
                            :, img_loc, h0 + 1:h0 + 5, 1:33]
                        src = tp[:, :128].rearrange("p (h w) -> p h w",
                                                    h=4, w=WW)
                        nc.scalar.activation(dst, src, AF.Copy)
                x3 = xpool.tile([96, IMG * HLEN], dt.float16, tag="x3", bufs=2)
                for i in range(ICAPS):
                    q, g = i % 2, i // 2
                    for dyi, dy in enumerate((-1, 0, 1)):
                        src = xs[i * 16:(i + 1) * 16,
                                 GUARD + dy * PW:GUARD + dy * PW + HLEN]
                        dst = x3[q * 48 + dyi * 16:q * 48 + dyi * 16 + 16,
                                 g * HLEN:(g + 1) * HLEN]
                        nc.sync.dma_start(dst, src)
                x3h.append(x3)

            # ---------------- per-chunk conv + routing --------------------
            st = [dict() for _ in range(NCHUNK)]

            def conv_S(ch):
                img, halfsel = ch // 2, ch % 2
                h0 = 1 + 16 * halfsel  # padded h of first output row
                half, img_loc = img // 2, img % 2

                def x3_rhs(i, dx):
                    g = i // 2
                    v = x3h[half][:, g * HLEN:(g + 1) * HLEN]
                    v = v.rearrange("p (im h w) -> p im h w", im=2, h=PH, w=PW)
                    return v[:, img_loc, h0:h0 + 16, 1 + dx:33 + dx]

                def xsb_rhs(dy, dx):
                    v = xgeom2(x_sbh[half][:, GUARD:GUARD + HLEN])
                    return v[:, img_loc, h0 + dy:h0 + dy + 16, 1 + dx:33 + dx]

                votes16 = vpool.tile([128, ICAPS * 512], dt.float16,
                                     tag="votes", bufs=3)
                v3 = votes16[:].rearrange("p (i n) -> p i n", i=ICAPS)
                # transposed votes [p, sub, i, (f,c)]
                votes_t = vpool.tile([128, 4 * ICAPS * 128], dt.float16,
                                     tag="votes_t", bufs=3)
                vt4 = votes_t[:].rearrange("p (s i n) -> p s i n",
                                           s=4, i=ICAPS)
                for i in range(ICAPS):
                    vp = ps_vp.tile([128, 512], dt.float32, tag="vp")
                    vps = vp[:].rearrange("p (h w) -> p h w", h=16, w=WW)
                    for dxi, dx in enumerate((-1, 0, 1)):
                        lhsT = cst["w_c"][:, (i * 3 + dxi) * CF:
                                          (i * 3 + dxi + 1) * CF]
                        nc.tensor.matmul(vps, lhsT, x3_rhs(i, dx),
                                         start=(dxi == 0), stop=(dxi == 2))
                    nc.scalar.activation(v3[:, i, :], vp[:], AF.Copy)
                    if i % 2 == 1:
                        # transpose this capsule pair's votes into votes_t
                        vt = ps_t16.tile([128, 1024], dt.float16, tag="t16")
                        for iq in range(2):
                            for sub in range(4):
                                nc.tensor.transpose(
                                    vt[:, iq * 512 + sub * 128:
                                       iq * 512 + (sub + 1) * 128],
                                    v3[:, i - 1 + iq,
                                       sub * 128:(sub + 1) * 128],
                                    cst["idm16"][:])
                        dstv = vt4[:, :, i - 1:i + 1, :]
                        srcv = vt[:].rearrange("p (i s n) -> p s i n",
                                               i=2, s=4)
                        nc.vector.tensor_copy(dstv, srcv)

                # S = (1/8) sum_i votes  (w_s pre-scaled)
                S = ps_vp.tile([128, 512], dt.float32, tag="vp")
                Ss = S[:].rearrange("p (h w) -> p h w", h=16, w=WW)
                for t, (dy, dx) in enumerate(TAPS):
                    nc.tensor.matmul(Ss, cst["w_s"][:, t * CF:(t + 1) * CF],
                                     xsb_rhs(dy, dx),
                                     start=(t == 0), stop=(t == 8))
                s16 = mpool.tile([128, 512], dt.float16, tag="s16")
                nc.scalar.activation(s16[:], S[:], AF.Copy)
                stp = ps_t16.tile([128, 1024], dt.float16, tag="t16")
                for sub in range(4):
                    nc.tensor.transpose(stp[:, sub * 128:(sub + 1) * 128],
                                        s16[:, sub * 128:(sub + 1) * 128],
                                        cst["idm16"][:])
                st16 = mpool.tile([128, 512], dt.float16, tag="st16")
                nc.scalar.activation(st16[:], stp[:, :512], AF.Copy)
                st[ch]["votes16"] = votes16
                st[ch]["v3"] = v3
                st[ch]["votes_t"] = votes_t
                st[ch]["vt4"] = vt4
                st[ch]["st16"] = st16

            def bview(n):
                return cst["b_fc"][:].unsqueeze(1).broadcast_to([128, n, CF])

            def squash_T(ch, pre_b, atag):
                """pre_b: [128, 512] fp16 SBUF viewed [p,(sub,f,c)].
                Returns act_t fp16 SBUF same shape."""
                sq = mpool.tile([128, 512], dt.float16, tag="sq", bufs=3)
                nc.scalar.activation(sq[:], pre_b[:], AF.Square)
                s2 = mpool.tile([128, 32], dt.float32, tag="s2", bufs=3)
                sqv = sq[:].rearrange("p (s f c) -> p s c f", s=4, f=F)
                nc.vector.tensor_reduce(
                    s2[:].rearrange("p (s c) -> p s c", s=4), sqv,
                    axis=AX.X, op=OP.add)
                yt = mpool.tile([128, 32], dt.float32, tag="yt", bufs=3)
                nc.scalar.activation(yt[:], s2[:], AF.Ln,
                                     bias=cst["epsv"][:])
                tq = mpool.tile([128, 32], dt.float32, tag="tq", bufs=3)
                nc.scalar.activation(tq[:], yt[:], AF.Exp, scale=0.5)
                uq = mpool.tile([128, 32], dt.float32, tag="uq", bufs=3)
                nc.vector.scalar_tensor_tensor(uq[:], s2[:], 1.0, tq[:],
                                               OP.add, OP.mult)
                rq = mpool.tile([128, 32], dt.float32, tag="rq", bufs=3)
                nc.vector.reciprocal_approx_fast(rq[:], uq[:])
                wq = mpool.tile([128, 32], dt.float16, tag="wq", bufs=3)
                nc.vector.tensor_tensor(wq[:], s2[:], rq[:], OP.mult)
                act = mpool.tile([128, 512], dt.float16, tag=atag, bufs=4)
                wv = wq[:].rearrange("p (s c) -> p s c", s=4)
                wb = wv.unsqueeze(2).broadcast_to([128, 4, F, C])
                av = act[:].rearrange("p (s f c) -> p s f c", s=4, f=F)
                pv = pre_b[:].rearrange("p (s f c) -> p s f c", s=4, f=F)
                nc.vector.tensor_tensor(av, pv, wb, OP.mult)
                return act

            def act_transpose(ch, act, tag):
                """act_t [p,(sub,fc)] SBUF -> act_o [(f,c), pos] SBUF fp16."""
                at = ps_t16.tile([128, 1024], dt.float16, tag="t16")
                for sub in range(4):
                    nc.tensor.transpose(at[:, sub * 128:(sub + 1) * 128],
                                        act[:, sub * 128:(sub + 1) * 128],
                                        cst["idm16"][:])
                ao = mpool.tile([128, 512], dt.float16, tag=tag, bufs=3)
                nc.vector.tensor_copy(ao[:], at[:, :512])
                return ao

            def iter1(ch):
                st16 = st[ch]["st16"]
                pre1 = mpool.tile([128, 512], dt.float16, tag="pre1")
                nc.vector.tensor_tensor(
                    pre1[:].rearrange("p (s n) -> p s n", s=4),
                    st16[:].rearrange("p (s n) -> p s n", s=4),
                    bview(4), OP.add)
                act = squash_T(ch, pre1, "act1")
                st[ch]["act_o"] = act_transpose(ch, act, "ao1")
                lz = ps_L.tile([64, 512], dt.float32, tag="L")
                st[ch]["L"] = lz

            def riter(ch, it):
                v3 = st[ch]["v3"]
                at = st[ch]["act_o"]
                L = st[ch]["L"][:]
                rv = wpool.tile([128, ICAPS * 512], dt.float16, tag="rv")
                rv3 = rv[:].rearrange("p (i n) -> p i n", i=ICAPS)
                at_b = at[:].unsqueeze(1).broadcast_to([128, ICAPS, 512])
                nc.vector.tensor_tensor(rv3, v3, at_b, OP.mult)
                for i in range(ICAPS):
                    nc.tensor.matmul(L, cst["ei"][:, i * 64:(i + 1) * 64],
                                     rv3[:, i, :],
                                     start=(it == 2 and i == 0),
                                     stop=(it == 3 and i == ICAPS - 1),
                                     skip_group_check=True)
                ev = mpool.tile([64, 512], dt.float32r, tag="ev")
                nc.scalar.activation(ev[:], L, AF.Exp,
                                     bias=cst["neg2"][:64, :])
                Z = zslice(ch)
                nc.tensor.matmul(Z, cst["oneii"][:], ev[:],
                                 start=True, stop=True)
                rz = mpool.tile([64, 512], dt.float32, tag="rz")
                nc.vector.reciprocal_approx_fast(rz[:], Z)
                route16 = mpool.tile([64, 512], dt.float16, tag="route16")
                nc.vector.tensor_tensor(route16[:], ev[:].bitcast(dt.float32),
                                        rz[:], OP.mult)
                rtp = ps_t16.tile([128, 1024], dt.float16, tag="t16")
                for sub in range(4):
                    nc.tensor.transpose(rtp[:, sub * 64:(sub + 1) * 64],
                                        route16[:, sub * 128:(sub + 1) * 128],
                                        cst["idm16"][:64, :64])
                rt = mpool.tile([128, 256], dt.float16, tag="rt_sb")
                nc.vector.tensor_copy(rt[:], rtp[:, :256])

                vt4 = st[ch]["vt4"]
                vt5 = st[ch]["votes_t"][:].rearrange(
                    "p (s i f c) -> p s i f c", s=4, i=ICAPS, f=F)
                rb = wpool.tile([128, ICAPS * 512], dt.float16, tag="rb")
                rb5 = rb[:].rearrange("p (s i f c) -> p s i f c",
                                      s=4, i=ICAPS, f=F)
                rtv = rt[:].rearrange("p (s i c) -> p s i c", s=4, i=ICAPS)
                rtb = rtv.unsqueeze(3).broadcast_to([128, 4, ICAPS, F, C])
                nc.vector.tensor_tensor(rb5, vt5, rtb, OP.mult)
                # i-sum tree: 8 -> 4 (gpsimd) -> 2 -> 1 (+bias)
                rb4 = rb[:].rearrange("p (s i n) -> p s i n", s=4, i=ICAPS)
                t1 = wpool.tile([128, 4 * 4 * 128], dt.float16, tag="t1")
                t14 = t1[:].rearrange("p (s i n) -> p s i n", s=4, i=4)
                nc.vector.tensor_tensor(t14, rb4[:, :, 0:4, :],
                                        rb4[:, :, 4:8, :], OP.add)
                t2 = wpool.tile([128, 4 * 2 * 128], dt.float16, tag="t2")
                t24 = t2[:].rearrange("p (s i n) -> p s i n", s=4, i=2)
                nc.vector.tensor_tensor(t24, t14[:, :, 0:2, :],
                                        t14[:, :, 2:4, :], OP.add)
                pre_raw = mpool.tile([128, 512], dt.float16, tag="pre_raw")
                pr4 = pre_raw[:].rearrange("p (s n) -> p s n", s=4)
                nc.vector.tensor_tensor(pr4, t24[:, :, 0, :], t24[:, :, 1, :],
                                        OP.add)
                pre_b = mpool.tile([128, 512], dt.float16, tag="pre_b")
                nc.vector.tensor_tensor(
                    pre_b[:].rearrange("p (s n) -> p s n", s=4),
                    pr4, bview(4), OP.add)
                if it == 2:
                    act = squash_T(ch, pre_b, "act2")
                    st[ch]["act_o"] = act_transpose(ch, act, "ao2")
                else:
                    st[ch]["act_f"] = squash_T(ch, pre_b, "actf")

            def out_chunk(ch):
                act_f = st[ch]["act_f"]
                dst = out_d[ch * 512:(ch + 1) * 512, :].rearrange(
                    "(s p) c -> p s c", p=128)
                nc.sync.dma_start(dst, act_f[:].rearrange(
                    "p (s c) -> p s c", s=4))
                st[ch].clear()

            # 3-deep skewed software pipeline
            for t in range(NCHUNK + 2):
                if t < NCHUNK:
                    conv_S(t)
                    iter1(t)
                if 1 <= t <= NCHUNK:
                    riter(t - 1, 2)
                if t >= 2:
                    riter(t - 2, 3)
                    out_chunk(t - 2)

    nc.compile()
    return nc


def kernel(input_tensor, W, b):
    x = np.ascontiguousarray(np.asarray(input_tensor, np.float32))
    B = x.shape[0]
    per = B // N_CORES
    assert x.shape == (32, 32, 32, 8, 16) and per == IMG

    if "nc" not in _CACHE:
        _CACHE["nc"] = _build_program()
    nc = _CACHE["nc"]

    cst = _host_constants(W, b)
    in_maps = []
    for core in range(N_CORES):
        shard = x[core * per:(core + 1) * per].reshape(IMG * HH * WW, 128)
        m = {"x": np.ascontiguousarray(shard)}
        m.update(cst)
        in_maps.append(m)
    res = run_bass_kernel_spmd(nc, in_maps, list(range(N_CORES)))
    # out columns are (f,c); permute back to (c,f)
    out = np.concatenate(
        [res.results[c]["out"].reshape(IMG, HH, WW, F, C)
         .transpose(0, 1, 2, 4, 3)
         for c in range(N_CORES)], axis=0)
    return np.ascontiguousarray(out).astype(np.float32)
